# revision 1
# baseline (speedup 1.0000x reference)
"""ConvAttention (GroupNorm + channel attention + residual) on 8 Trainium2
NeuronCores, data-parallel over the batch dimension (B=8 -> 1 item/core).

Per-core algorithm (x is (C=512, N=4096) for one batch item):
  1. GroupNorm(32 groups) via per-channel bn_stats + tiny indicator matmuls
     for the cross-partition group reduction / broadcast; the affine
     (x*a + b) runs on the Scalar engine, producing g in bf16.
  2. scores = q k^T (contracted over N) is computed via the Gram matrix:
         scores = Wq (g g^T) Wk^T + (Wq sg) (x) bk + bq (x) (Wk sg + N bk)
     with sg = row sums of g.  g g^T needs g^T, produced by PE transposes
     that are pipelined with the Gram accumulation.
  3. softmax rows, fused: exp((s - max) * alpha) on the Scalar engine with
     accumulated row sums; probs scaled by 1/sum, then PE-transposed.
  4. attn^T = g^T M^T + 1 (x) (bv^T probs^T), with M^T = Wv^T probs^T.
     attn^T lands in (N, C) layout, which makes the reference's
     permute+reshape a flat-memory no-op:  out.flat = x.flat + attnT.flat.
  5. Residual x is re-streamed into the (N, C) tile layout by DMA and the
     final add is fused into the PSUM evacuation.
"""
import sys

if "/opt/trn_rl_repo" not in sys.path:
    sys.path.insert(0, "/opt/trn_rl_repo")

from contextlib import ExitStack

import ml_dtypes
import numpy as np

import concourse.bass as bass
import concourse.tile as tile
from concourse import bacc, mybir
from concourse import bass_utils
from concourse.masks import make_identity

BF16 = ml_dtypes.bfloat16
bf = mybir.dt.bfloat16
f32 = mybir.dt.float32

B, C, H, W = 8, 512, 64, 64
N = H * W            # 4096 spatial tokens
GROUPS = 32
GS = C // GROUPS     # 16 channels per group
EPS = 1e-6
ALPHA = float(C) ** -0.5
P = 128
CT = C // P          # 4 channel tiles
NT = N // P          # 32 spatial tiles
SUB = 512            # bn_stats subgroup width
NSUB = N // SUB      # 8

AF = mybir.ActivationFunctionType
AX = mybir.AxisListType
OP = mybir.AluOpType


def _build_program():
    nc = bacc.Bacc("TRN2", target_bir_lowering=False, debug=False, num_devices=B)

    x_d = nc.dram_tensor("x", (C, N), f32, kind="ExternalInput").ap()
    wqT_d = nc.dram_tensor("wqT", (C, C), bf, kind="ExternalInput").ap()
    wkT_d = nc.dram_tensor("wkT", (C, C), bf, kind="ExternalInput").ap()
    wv_d = nc.dram_tensor("wv", (C, C), bf, kind="ExternalInput").ap()
    bqr_d = nc.dram_tensor("bq_row", (1, C), bf, kind="ExternalInput").ap()
    bkr_d = nc.dram_tensor("bk_row", (1, C), bf, kind="ExternalInput").ap()
    bk4_d = nc.dram_tensor("bk_n", (1, C), f32, kind="ExternalInput").ap()
    bv_d = nc.dram_tensor("bv_col", (C, 1), bf, kind="ExternalInput").ap()
    gnw_d = nc.dram_tensor("gnw", (C, 1), f32, kind="ExternalInput").ap()
    gnb_d = nc.dram_tensor("gnb", (C, 1), f32, kind="ExternalInput").ap()
    i16_d = nc.dram_tensor("ind16", (C, 8), f32, kind="ExternalInput").ap()
    iT_d = nc.dram_tensor("indT01", (8, P), f32, kind="ExternalInput").ap()
    out_d = nc.dram_tensor("out", (N, C), f32, kind="ExternalOutput").ap()

    with tile.TileContext(nc) as tc, ExitStack() as ctx:
        consts = ctx.enter_context(tc.tile_pool(name="consts", bufs=1))
        px = ctx.enter_context(tc.tile_pool(name="px", bufs=1))
        pg = ctx.enter_context(tc.tile_pool(name="pg", bufs=1))
        pmats = ctx.enter_context(tc.tile_pool(name="pmats", bufs=1))
        pgt = ctx.enter_context(tc.tile_pool(name="pgt", bufs=3))
        psmall = ctx.enter_context(tc.tile_pool(name="psmall", bufs=4))
        presid = ctx.enter_context(tc.tile_pool(name="presid", bufs=6))
        pout = ctx.enter_context(tc.tile_pool(name="pout", bufs=6))
        # PSUM: 4 (gram accum) + 2 (transpose) + 2 (rotating matmul out) = 8 banks
        ps_big = ctx.enter_context(tc.tile_pool(name="ps_big", bufs=2, space="PSUM"))
        ps_ctx = ExitStack()
        ps_acc = ps_ctx.enter_context(tc.tile_pool(name="ps_acc", bufs=1, space="PSUM"))
        ps_tr = ps_ctx.enter_context(tc.tile_pool(name="ps_tr", bufs=2, space="PSUM"))

        # ---------------- constants / weights ----------------
        ident = consts.tile([P, P], bf, tag="ident")
        make_identity(nc, ident)

        wq_sb, wk_sb, wv_sb = [], [], []
        for t in range(CT):
            wq_t = consts.tile([P, C], bf, tag=f"wq{t}")
            nc.gpsimd.dma_start(wq_t, wqT_d[t * P:(t + 1) * P, :])
            wq_sb.append(wq_t)
            wk_t = consts.tile([P, C], bf, tag=f"wk{t}")
            nc.gpsimd.dma_start(wk_t, wkT_d[t * P:(t + 1) * P, :])
            wk_sb.append(wk_t)
            wv_t = consts.tile([P, C], bf, tag=f"wv{t}")
            nc.gpsimd.dma_start(wv_t, wv_d[t * P:(t + 1) * P, :])
            wv_sb.append(wv_t)

        bqr = consts.tile([1, C], bf, tag="bqr")
        nc.gpsimd.dma_start(bqr, bqr_d)
        bkr = consts.tile([1, C], bf, tag="bkr")
        nc.gpsimd.dma_start(bkr, bkr_d)
        bk4 = consts.tile([1, C], f32, tag="bk4")
        nc.gpsimd.dma_start(bk4, bk4_d)

        bv_sb, gnw_sb, gnb_sb, i16_sb = [], [], [], []
        for t in range(CT):
            bv_t = consts.tile([P, 1], bf, tag=f"bv{t}")
            nc.gpsimd.dma_start(bv_t, bv_d[t * P:(t + 1) * P, :])
            bv_sb.append(bv_t)
            gnw_t = consts.tile([P, 1], f32, tag=f"gnw{t}")
            nc.gpsimd.dma_start(gnw_t, gnw_d[t * P:(t + 1) * P, :])
            gnw_sb.append(gnw_t)
            gnb_t = consts.tile([P, 1], f32, tag=f"gnb{t}")
            nc.gpsimd.dma_start(gnb_t, gnb_d[t * P:(t + 1) * P, :])
            gnb_sb.append(gnb_t)
            i16_t = consts.tile([P, 8], f32, tag=f"i16{t}")
            nc.gpsimd.dma_start(i16_t, i16_d[t * P:(t + 1) * P, :])
            i16_sb.append(i16_t)
        iT_sb = consts.tile([8, P], f32, tag="iT")
        nc.gpsimd.dma_start(iT_sb, iT_d)
        eps8 = consts.tile([8, 1], f32, tag="eps8")
        nc.vector.memset(eps8, EPS)
        ones1 = consts.tile([1, P], bf, tag="ones1")
        nc.vector.memset(ones1, 1.0)

        # ---------------- phase 1+2: per-tile GroupNorm (fully pipelined) ------
        # Groups (16ch) never cross a 128-channel tile, so each tile reduces its
        # own 8 groups: stats -> local indicator matmuls -> affine, no barrier.
        dmae = [nc.sync, nc.scalar]  # alternate the two HWDGE queues
        x_sb, g_sb, sg_sb = [], [], []
        for ci in range(CT):
            x_t = px.tile([P, N], f32, tag=f"x{ci}")
            last = nc.sync if ci % 2 == 0 else nc.scalar
            for q, eng in enumerate([nc.sync, nc.scalar, nc.gpsimd, last]):
                sl = slice(q * N // 4, (q + 1) * N // 4)
                eng.dma_start(x_t[:, sl], x_d[ci * P:(ci + 1) * P, sl])
            x_sb.append(x_t)

            stats = psmall.tile([P, NSUB, 6], f32, tag="stats")
            xv = x_t.rearrange("p (s f) -> p s f", f=SUB)
            for s in range(NSUB):
                nc.vector.bn_stats(out=stats[:, s, :], in_=xv[:, s, :])
            mv = psmall.tile([P, 2], f32, tag=f"mv{ci}", bufs=1)
            nc.vector.bn_aggr(out=mv, in_=stats)

            # st2 = [mu_p, E[x^2]_p] per channel
            st2 = psmall.tile([P, 2], f32, tag="st2")
            nc.vector.tensor_copy(st2[:, 0:1], mv[:, 0:1])
            e2 = psmall.tile([P, 1], f32, tag="e2")
            nc.vector.tensor_scalar(e2, mv[:, 0:1], mv[:, 0:1], None, op0=OP.mult)
            nc.vector.tensor_tensor(st2[:, 1:2], e2, mv[:, 1:2], OP.add)
            # local 8-group reduction (1/16-weighted indicator)
            gst = ps_big.tile([8, 2], f32, tag="big")
            nc.tensor.matmul(gst, lhsT=i16_sb[ci], rhs=st2, start=True, stop=True)

            gtmp = psmall.tile([8, 1], f32, tag="gtmp")
            nc.vector.tensor_scalar(gtmp, gst[:, 0:1], gst[:, 0:1], None, op0=OP.mult)
            gvar = psmall.tile([8, 1], f32, tag="gvar")
            nc.vector.tensor_tensor(gvar, gst[:, 1:2], gtmp, OP.subtract)
            gsd = psmall.tile([8, 1], f32, tag="gsd")
            nc.scalar.activation(gsd, gvar, AF.Sqrt, bias=eps8, scale=1.0)
            grs = psmall.tile([8, 1], f32, tag="grs")
            nc.vector.reciprocal(grs, gsd)
            gr2 = psmall.tile([8, 2], f32, tag="gr2")
            nc.vector.tensor_copy(gr2[:, 0:1], gst[:, 0:1])
            nc.vector.tensor_copy(gr2[:, 1:2], grs)

            bc = ps_big.tile([P, 2], f32, tag="big")
            nc.tensor.matmul(bc, lhsT=iT_sb, rhs=gr2, start=True, stop=True)
            a_col = psmall.tile([P, 1], f32, tag=f"a{ci}", bufs=1)
            nc.vector.tensor_tensor(a_col, gnw_sb[ci], bc[:, 1:2], OP.mult)
            tmp = psmall.tile([P, 1], f32, tag="tmp")
            nc.vector.tensor_tensor(tmp, bc[:, 0:1], a_col, OP.mult)
            b_col = psmall.tile([P, 1], f32, tag=f"b{ci}", bufs=1)
            nc.vector.tensor_tensor(b_col, gnb_sb[ci], tmp, OP.subtract)

            g_t = pg.tile([P, N], bf, tag=f"g{ci}")
            if ci % 2 == 0:
                # Scalar engine: g = Identity(x * a + b)
                nc.scalar.activation(g_t, x_sb[ci], AF.Identity,
                                     bias=b_col, scale=a_col)
            else:
                # Vector engine: same affine, keeps both engines busy
                nc.vector.tensor_scalar(g_t, x_sb[ci], a_col, b_col,
                                        op0=OP.mult, op1=OP.add)
            g_sb.append(g_t)

            # sg = N * (a * mu_p + b)  (row sums of g), as bf16 column
            t2 = psmall.tile([P, 1], f32, tag="t2")
            nc.vector.tensor_tensor(t2, a_col, mv[:, 0:1], OP.mult)
            nc.vector.tensor_tensor(t2, t2, b_col, OP.add)
            sg_t = consts.tile([P, 1], bf, tag=f"sg{ci}")
            nc.vector.tensor_scalar(sg_t, t2, float(N), None, op0=OP.mult)
            sg_sb.append(sg_t)

        # ---------------- phase 3: Gram = g g^T, pipelined with g^T transposes ----------------
        G_ps = [ps_acc.tile([P, C], f32, tag=f"G{i}", name=f"Gps{i}")
                for i in range(CT)]
        # NOTE: tag G0 reused after gstats is fully consumed above.
        prev_gt = None
        for nt in range(NT + 1):
            if nt < NT:
                trp = ps_tr.tile([P, C], bf, tag="tr")
                for it in range(CT):
                    nc.tensor.transpose(trp[:, it * P:(it + 1) * P],
                                        g_sb[it][:, nt * P:(nt + 1) * P], ident)
                gt = pgt.tile([P, C], bf, tag="gt")
                nc.vector.tensor_copy(gt, trp)
            else:
                gt = None
            if prev_gt is not None:
                for io in range(CT):
                    nc.tensor.matmul(G_ps[io], lhsT=prev_gt[:, io * P:(io + 1) * P],
                                     rhs=prev_gt, start=(nt == 1), stop=(nt == NT))
            prev_gt = gt

        G_sb = []
        for io in range(CT):
            G_t = pmats.tile([P, C], bf, tag=f"Gm{io}")
            nc.vector.tensor_copy(G_t, G_ps[io])
            G_sb.append(G_t)

        # ---------------- phase 4: bias rows t1 = Wq sg, u = Wk sg + N bk ----------------
        t1p = ps_big.tile([1, C], f32, tag="big")
        for it in range(CT):
            nc.tensor.matmul(t1p, lhsT=sg_sb[it], rhs=wq_sb[it],
                             start=(it == 0), stop=(it == CT - 1))
        t2p = ps_big.tile([1, C], f32, tag="big")
        for it in range(CT):
            nc.tensor.matmul(t2p, lhsT=sg_sb[it], rhs=wk_sb[it],
                             start=(it == 0), stop=(it == CT - 1))
        t1row = consts.tile([1, C], bf, tag="t1row")
        nc.vector.tensor_copy(t1row, t1p)
        urow = consts.tile([1, C], bf, tag="urow")
        nc.vector.tensor_tensor(urow, t2p, bk4, OP.add)

        # ------- phase 5: A^T = G Wq^T directly (G is symmetric) -------
        AT_sb = []
        for jt in range(CT):
            Ap = ps_big.tile([P, C], f32, tag="big")
            for it in range(CT):
                nc.tensor.matmul(Ap, lhsT=G_sb[it][:, jt * P:(jt + 1) * P],
                                 rhs=wq_sb[it], start=(it == 0), stop=(it == CT - 1))
            AT_t = pmats.tile([P, C], bf, tag=f"AT{jt}", name=f"ATsb{jt}")
            nc.vector.tensor_copy(AT_t, Ap)
            AT_sb.append(AT_t)

        # ---------------- phase 6: scores + softmax ----------------
        pr_sb = []
        for ct in range(CT):
            scp = ps_big.tile([P, C], f32, tag="big")
            for jt in range(CT):
                nc.tensor.matmul(scp, lhsT=AT_sb[jt][:, ct * P:(ct + 1) * P],
                                 rhs=wk_sb[jt], start=(jt == 0), stop=False)
            nc.tensor.matmul(scp, lhsT=t1row[0:1, ct * P:(ct + 1) * P], rhs=bkr,
                             start=False, stop=False)
            nc.tensor.matmul(scp, lhsT=bqr[0:1, ct * P:(ct + 1) * P], rhs=urow,
                             start=False, stop=True)
            nm = psmall.tile([P, 1], f32, tag="nm")
            nc.vector.reduce_max(nm, scp, axis=AX.X, negate=True)
            nma = psmall.tile([P, 1], f32, tag="nma")
            nc.vector.tensor_scalar(nma, nm, ALPHA, None, op0=OP.mult)
            se = psmall.tile([P, 1], f32, tag="se")
            pr_t = pmats.tile([P, C], bf, tag=f"pr{ct}")
            nc.scalar.activation(pr_t, scp, AF.Exp, bias=nma, scale=ALPHA,
                                 accum_out=se)
            ri = psmall.tile([P, 1], f32, tag="ri")
            nc.vector.reciprocal(ri, se)
            nc.vector.tensor_scalar_mul(pr_t, pr_t, ri)
            pr_sb.append(pr_t)

        # probs^T
        prT_sb = [pmats.tile([P, C], bf, tag=f"prT{dt}", name=f"prTsb{dt}")
                  for dt in range(CT)]
        for ct in range(CT):
            trp = ps_tr.tile([P, C], bf, tag="tr")
            for dt in range(CT):
                nc.tensor.transpose(trp[:, dt * P:(dt + 1) * P],
                                    pr_sb[ct][:, dt * P:(dt + 1) * P], ident)
            for dt in range(CT):
                nc.vector.tensor_copy(prT_sb[dt][:, ct * P:(ct + 1) * P],
                                      trp[:, dt * P:(dt + 1) * P])

        # ---------------- phase 7: M^T = Wv^T probs^T, pv row ----------------
        MT_sb = []
        for it in range(CT):
            Mp = ps_big.tile([P, C], f32, tag="big")
            for dt in range(CT):
                nc.tensor.matmul(Mp, lhsT=wv_sb[dt][:, it * P:(it + 1) * P],
                                 rhs=prT_sb[dt], start=(dt == 0), stop=(dt == CT - 1))
            MT_t = pmats.tile([P, C], bf, tag=f"MT{it}")
            nc.vector.tensor_copy(MT_t, Mp)
            MT_sb.append(MT_t)

        pvp = ps_big.tile([1, C], f32, tag="big")
        for dt in range(CT):
            nc.tensor.matmul(pvp, lhsT=bv_sb[dt], rhs=prT_sb[dt],
                             start=(dt == 0), stop=(dt == CT - 1))
        pvrow = consts.tile([1, C], bf, tag="pvrow")
        nc.vector.tensor_copy(pvrow, pvp)

        # ------- phase 8: attn^T (n, c) + residual + store -------
        # attnT[n, c] = sum_i g[i, n] M[c, i] + pv[c]; flat (n, c) order equals
        # the reference's permute+reshape flat order, so out rows store
        # contiguously and the residual x streams in via a reshape DMA.
        ps_ctx.close()  # release gram/transpose banks
        ps_att = ctx.enter_context(tc.tile_pool(name="ps_att", bufs=4, space="PSUM"))
        for nt in range(NT):
            at = ps_att.tile([P, C], f32, tag="att", name=f"at{nt}")
            for it in range(CT):
                nc.tensor.matmul(at, lhsT=g_sb[it][:, nt * P:(nt + 1) * P],
                                 rhs=MT_sb[it], start=(it == 0), stop=False)
            nc.tensor.matmul(at, lhsT=ones1, rhs=pvrow, start=False, stop=True)

            resid = presid.tile([P, C], f32, tag="resid")
            ci, lo = nt // 8, nt % 8
            nc.scalar.dma_start(
                resid,
                x_sb[ci][16 * lo:16 * (lo + 1), :].rearrange(
                    "p (u f) -> p u f", u=8),
            )
            osb = pout.tile([P, C], f32, tag="o")
            nc.vector.tensor_tensor(osb, at, resid, OP.add)
            nc.sync.dma_start(out_d[nt * P:(nt + 1) * P, :], osb)

    nc.compile()
    return nc


_NC = None


def _get_program():
    global _NC
    if _NC is None:
        _NC = _build_program()
    return _NC


def _stage_inputs(x, gn_w, gn_b, wq, bq, wk, bk, wv, bv):
    """Build the per-core input maps (host-side sharding / layout prep)."""
    x = np.asarray(x, dtype=np.float32).reshape(B, C, N)
    shared = {
        "wqT": np.ascontiguousarray(np.asarray(wq, np.float32).T).astype(BF16),
        "wkT": np.ascontiguousarray(np.asarray(wk, np.float32).T).astype(BF16),
        "wv": np.ascontiguousarray(np.asarray(wv, np.float32)).astype(BF16),
        "bq_row": np.asarray(bq, np.float32).reshape(1, C).astype(BF16),
        "bk_row": np.asarray(bk, np.float32).reshape(1, C).astype(BF16),
        "bk_n": (float(N) * np.asarray(bk, np.float32)).reshape(1, C),
        "bv_col": np.asarray(bv, np.float32).reshape(C, 1).astype(BF16),
        "gnw": np.asarray(gn_w, np.float32).reshape(C, 1),
        "gnb": np.asarray(gn_b, np.float32).reshape(C, 1),
    }
    ind16 = np.zeros((C, 8), np.float32)
    indT = np.zeros((8, P), np.float32)
    for c in range(C):
        ind16[c, (c % P) // GS] = 1.0 / GS
    for p in range(P):
        indT[p // GS, p] = 1.0
    shared["ind16"] = ind16
    shared["indT01"] = indT

    in_maps = []
    for b in range(B):
        m = dict(shared)
        m["x"] = np.ascontiguousarray(x[b])
        in_maps.append(m)
    return in_maps


def kernel(x, gn_w, gn_b, wq, bq, wk, bk, wv, bv, _trace=False, _tmpdir=None):
    nc = _get_program()
    in_maps = _stage_inputs(x, gn_w, gn_b, wq, bq, wk, bk, wv, bv)
    res = bass_utils.run_bass_kernel_spmd(
        nc, in_maps, core_ids=list(range(B)), trace=_trace, tmpdir=_tmpdir,
    )
    out = np.stack([res.results[b]["out"].reshape(C, H, W) for b in range(B)])
    if _trace:
        kernel._last_results = res
    return out.astype(np.float32)



# revision 8
# speedup vs baseline: 1.4346x; 1.4346x over previous
"""ConvAttention (GroupNorm + channel attention + residual) on 8 Trainium2
NeuronCores, data-parallel over batch (B=8 -> 1 item/core).

v2: GroupNorm is folded into the attention algebra so nothing on the
critical path waits for it and g is never materialized:

  g = D x + beta 1^T          (D = diag(a), per-channel affine from stats)
  Gg = D Gx D + rank-1s       (Gx = x x^T accumulates while x streams in)
  scores = Wq D Gx D Wk^T + qp kb^T + qb ks^T + qs bk^T + bq u^T
  attn^T = x^T (D M^T) + 1 (x) (vb^T probs^T),  M^T = Wv^T probs^T,
           vb = Wv beta + bv

Per-core pipeline:
  1. DMA in: xt2 (x^T, fp8, DoubleRow-interleaved pairs) feeds the Gram
     accumulation on the PE at 2 k-tiles/instr; xb (x, bf16) feeds
     bn_stats on Vector and is cast to fp8 (xf2) for the attention lhsT.
  2. After stats: a,beta via tiny indicator matmuls; matvec rows for the
     four rank-1 score corrections (stacked into one (4,C) matmul).
  3. Gs = a*Gx evac -> A^T = Gs^T Wq^T -> scores (+rank-1 stack) ->
     fused softmax -> probs^T via PE transposes -> M^T -> MT' = a*M^T
     (fp8) -> pvrow.
  4. attn^T per 128-token tile: 2 fp8 DoubleRow matmuls + 1 DoubleRow
     rank-1; residual x streams from xb via flat-reinterpret DMA; final
     add fused into PSUM evacuation, stores fp32 (N, C) rows.
"""
import sys

if "/opt/trn_rl_repo" not in sys.path:
    sys.path.insert(0, "/opt/trn_rl_repo")

from contextlib import ExitStack

import ml_dtypes
import numpy as np

import concourse.bass as bass
import concourse.tile as tile
from concourse import bacc, mybir
from concourse import bass_utils
from concourse.masks import make_identity

BF16 = ml_dtypes.bfloat16
F8NP = ml_dtypes.float8_e4m3fn
bf = mybir.dt.bfloat16
f32 = mybir.dt.float32
f8 = mybir.dt.float8e4

B, C, H, W = 8, 512, 64, 64
N = H * W            # 4096 spatial tokens
GROUPS = 32
GS = C // GROUPS     # 16 channels per group
EPS = 1e-6
ALPHA = float(C) ** -0.5
P = 128
CT = C // P          # 4 channel tiles
NT = N // P          # 32 spatial tiles
NPAIR = NT // 2      # 16 DoubleRow token-pair tiles
SUB = 512            # bn_stats subgroup width
NSUB = N // SUB      # 8

AF = mybir.ActivationFunctionType
AX = mybir.AxisListType
OP = mybir.AluOpType
PM = mybir.MatmulPerfMode


def _build_program():
    nc = bacc.Bacc("TRN2", target_bir_lowering=False, debug=False, num_devices=B)

    xb_d = nc.dram_tensor("xb", (C, N), bf, kind="ExternalInput").ap()
    xt2_d = nc.dram_tensor("xt2", (NPAIR * P, 2 * C), f8, kind="ExternalInput").ap()
    wqT_d = nc.dram_tensor("wqT", (C, C), bf, kind="ExternalInput").ap()
    wkT_d = nc.dram_tensor("wkT", (C, C), bf, kind="ExternalInput").ap()
    wv_d = nc.dram_tensor("wv", (C, C), bf, kind="ExternalInput").ap()
    wvT_d = nc.dram_tensor("wvT", (C, C), bf, kind="ExternalInput").ap()
    bqr_d = nc.dram_tensor("bq_row", (1, C), bf, kind="ExternalInput").ap()
    bkr_d = nc.dram_tensor("bk_row", (1, C), bf, kind="ExternalInput").ap()
    bvr_d = nc.dram_tensor("bv_row", (1, C), f32, kind="ExternalInput").ap()
    lq2_d = nc.dram_tensor("lq2", (2, 4), bf, kind="ExternalInput").ap()
    lq1_d = nc.dram_tensor("lq1", (1, 4), bf, kind="ExternalInput").ap()
    lk2_d = nc.dram_tensor("lk2", (2, 4), bf, kind="ExternalInput").ap()
    lk1_d = nc.dram_tensor("lk1", (1, 4), bf, kind="ExternalInput").ap()
    gnw_d = nc.dram_tensor("gnw", (C, 1), f32, kind="ExternalInput").ap()
    gnb_d = nc.dram_tensor("gnb", (C, 1), f32, kind="ExternalInput").ap()
    i16_d = nc.dram_tensor("ind16", (C, 8), f32, kind="ExternalInput").ap()
    iT_d = nc.dram_tensor("indT01", (8, P), f32, kind="ExternalInput").ap()
    oq_d = nc.dram_tensor("oq", (1, 2 * P), f8, kind="ExternalInput").ap()
    one_d = nc.dram_tensor("one11", (1, 1), bf, kind="ExternalInput").ap()
    out_d = nc.dram_tensor("out", (N, C), f32, kind="ExternalOutput").ap()

    with tile.TileContext(nc) as tc, ExitStack() as ctx:
        consts = ctx.enter_context(tc.tile_pool(name="consts", bufs=1))
        px = ctx.enter_context(tc.tile_pool(name="px", bufs=1))
        pxt = ctx.enter_context(tc.tile_pool(name="pxt", bufs=1))
        pmats = ctx.enter_context(tc.tile_pool(name="pmats", bufs=1))
        psmall = ctx.enter_context(tc.tile_pool(name="psmall", bufs=4))
        presid = ctx.enter_context(tc.tile_pool(name="presid", bufs=6))
        pout = ctx.enter_context(tc.tile_pool(name="pout", bufs=6))
        # PSUM: gram 4 (inner ctx) + big rotating 2; later att 4 + tr 2
        ps_big = ctx.enter_context(tc.tile_pool(name="ps_big", bufs=2, space="PSUM"))
        ps_ctx = ExitStack()
        ps_gram = ps_ctx.enter_context(tc.tile_pool(name="ps_gram", bufs=1, space="PSUM"))

        # ---------------- constants / weights ----------------
        ident = consts.tile([P, P], bf, tag="ident")
        make_identity(nc, ident)

        wq_sb, wk_sb, wv_sb, wvT_sb = [], [], [], []
        for t in range(CT):
            w_t = consts.tile([P, C], bf, tag=f"wq{t}")
            nc.gpsimd.dma_start(w_t, wqT_d[t * P:(t + 1) * P, :])
            wq_sb.append(w_t)
            w_t = consts.tile([P, C], bf, tag=f"wk{t}")
            nc.gpsimd.dma_start(w_t, wkT_d[t * P:(t + 1) * P, :])
            wk_sb.append(w_t)
            w_t = consts.tile([P, C], bf, tag=f"wv{t}")
            nc.gpsimd.dma_start(w_t, wv_d[t * P:(t + 1) * P, :])
            wv_sb.append(w_t)
            w_t = consts.tile([P, C], bf, tag=f"wvT{t}")
            nc.gpsimd.dma_start(w_t, wvT_d[t * P:(t + 1) * P, :])
            wvT_sb.append(w_t)

        bqr = consts.tile([1, C], bf, tag="bqr")
        nc.gpsimd.dma_start(bqr, bqr_d)
        bkr = consts.tile([1, C], bf, tag="bkr")
        nc.gpsimd.dma_start(bkr, bkr_d)
        lq2 = consts.tile([2, 4], bf, tag="lq2")
        nc.gpsimd.dma_start(lq2, lq2_d)
        lq1 = consts.tile([1, 4], bf, tag="lq1")
        nc.gpsimd.dma_start(lq1, lq1_d)
        lk2 = consts.tile([2, 4], bf, tag="lk2")
        nc.gpsimd.dma_start(lk2, lk2_d)
        lk1 = consts.tile([1, 4], bf, tag="lk1")
        nc.gpsimd.dma_start(lk1, lk1_d)
        bvr = consts.tile([1, C], f32, tag="bvr")
        nc.gpsimd.dma_start(bvr, bvr_d)
        oq = consts.tile([1, 2 * P], f8, tag="oq")
        nc.gpsimd.dma_start(oq, oq_d)
        one11 = consts.tile([1, 1], bf, tag="one11")
        nc.gpsimd.dma_start(one11, one_d)

        gnw_sb, gnb_sb, i16_sb = [], [], []
        for t in range(CT):
            g_t = consts.tile([P, 1], f32, tag=f"gnw{t}")
            nc.gpsimd.dma_start(g_t, gnw_d[t * P:(t + 1) * P, :])
            gnw_sb.append(g_t)
            g_t = consts.tile([P, 1], f32, tag=f"gnb{t}")
            nc.gpsimd.dma_start(g_t, gnb_d[t * P:(t + 1) * P, :])
            gnb_sb.append(g_t)
            g_t = consts.tile([P, 8], f32, tag=f"i16{t}")
            nc.gpsimd.dma_start(g_t, i16_d[t * P:(t + 1) * P, :])
            i16_sb.append(g_t)
        iT_sb = consts.tile([8, P], f32, tag="iT")
        nc.gpsimd.dma_start(iT_sb, iT_d)
        eps8 = consts.tile([8, 1], f32, tag="eps8")
        nc.vector.memset(eps8, EPS)

        # ---------------- phase A: stream x in; Gram + stats ----------------
        # xt2 (PE food) and xb (stats food) interleave across the two
        # fastest queues so the Gram is never starved and stats finish
        # right after the loads do.
        xt2_sb = pxt.tile([P, NPAIR, 2 * C], f8, tag="xt2")
        x_sb = []
        for ci in range(CT):
            x_t = px.tile([P, N], f32 if False else bf, tag=f"x{ci}")
            x_sb.append(x_t)
        xt2v = xt2_d.rearrange("(j p) f -> p j f", p=P)
        # interleave: 8 xt2 slabs (2 pairs each), 8 xb halves
        for r in range(8):
            nc.sync.dma_start(xt2_sb[:, 2 * r:2 * r + 2, :],
                              xt2v[:, 2 * r:2 * r + 2, :])
            ci, h = r // 2, r % 2
            nc.scalar.dma_start(
                x_sb[ci][:, h * (N // 2):(h + 1) * (N // 2)],
                xb_d[ci * P:(ci + 1) * P, h * (N // 2):(h + 1) * (N // 2)])

        # Gram accumulation: G_ps[io] += sum_j xt2[:,j,io-blk]^T (x) xt2[:,j,:]
        G_ps = [ps_gram.tile([P, C], f32, tag=f"G{i}", name=f"Gps{i}")
                for i in range(CT)]
        for j in range(NPAIR):
            xt2j = xt2_sb[:, j, :].rearrange("p (i c) -> p i c", i=2)
            for io in range(CT):
                nc.tensor.matmul(G_ps[io], lhsT=xt2j[:, :, io * P:(io + 1) * P],
                                 rhs=xt2j, start=(j == 0), stop=(j == NPAIR - 1),
                                 perf_mode=PM.DoubleRow)

        # stats on xb (vector, overlaps the Gram)
        mv_sb = []
        xf2_sb = [pxt.tile([P, 2, N], f8, tag=f"xf{t}", name=f"xf2sb{t}")
                  for t in range(2)]
        for ci in range(CT):
            stats = psmall.tile([P, NSUB, 6], f32, tag="stats")
            xv = x_sb[ci].rearrange("p (s f) -> p s f", f=SUB)
            for s in range(NSUB):
                nc.vector.bn_stats(out=stats[:, s, :], in_=xv[:, s, :])
            mv = psmall.tile([P, 2], f32, tag=f"mv{ci}", bufs=1)
            nc.vector.bn_aggr(out=mv, in_=stats)
            mv_sb.append(mv)
            # cast x tile to fp8 for the attention lhsT
            nc.vector.tensor_copy(xf2_sb[ci // 2][:, ci % 2, :], x_sb[ci])

        # ---------------- group-norm coefficients ----------------
        a_sb, pb_sb, beta_sb = [], [], []
        for ci in range(CT):
            mv = mv_sb[ci]
            st2 = psmall.tile([P, 2], f32, tag="st2")
            nc.vector.tensor_copy(st2[:, 0:1], mv[:, 0:1])
            e2 = psmall.tile([P, 1], f32, tag="e2")
            nc.vector.tensor_scalar(e2, mv[:, 0:1], mv[:, 0:1], None, op0=OP.mult)
            nc.vector.tensor_tensor(st2[:, 1:2], e2, mv[:, 1:2], OP.add)
            gst = ps_big.tile([8, 2], f32, tag="big")
            nc.tensor.matmul(gst, lhsT=i16_sb[ci], rhs=st2, start=True, stop=True)
            gtmp = psmall.tile([8, 1], f32, tag="gtmp")
            nc.vector.tensor_scalar(gtmp, gst[:, 0:1], gst[:, 0:1], None, op0=OP.mult)
            gvar = psmall.tile([8, 1], f32, tag="gvar")
            nc.vector.tensor_tensor(gvar, gst[:, 1:2], gtmp, OP.subtract)
            gsd = psmall.tile([8, 1], f32, tag="gsd")
            nc.scalar.activation(gsd, gvar, AF.Sqrt, bias=eps8, scale=1.0)
            grs = psmall.tile([8, 1], f32, tag="grs")
            nc.vector.reciprocal(grs, gsd)
            gr2 = psmall.tile([8, 2], f32, tag="gr2")
            nc.vector.tensor_copy(gr2[:, 0:1], gst[:, 0:1])
            nc.vector.tensor_copy(gr2[:, 1:2], grs)
            bc = ps_big.tile([P, 2], f32, tag="big")
            nc.tensor.matmul(bc, lhsT=iT_sb, rhs=gr2, start=True, stop=True)
            a_col = psmall.tile([P, 1], f32, tag=f"a{ci}", bufs=1)
            nc.vector.tensor_tensor(a_col, gnw_sb[ci], bc[:, 1:2], OP.mult)
            tmp = psmall.tile([P, 1], f32, tag="tmp")
            nc.vector.tensor_tensor(tmp, bc[:, 0:1], a_col, OP.mult)
            b_col = psmall.tile([P, 1], f32, tag=f"b{ci}", bufs=1)
            nc.vector.tensor_tensor(b_col, gnb_sb[ci], tmp, OP.subtract)
            a_sb.append(a_col)
            beta_col = psmall.tile([P, 1], bf, tag=f"bb{ci}", bufs=1)
            nc.vector.tensor_copy(beta_col, b_col)
            beta_sb.append(beta_col)
            # p = a * sx = a * N * mu ; stacked [p beta] for the matvecs
            pb = psmall.tile([P, 2], bf, tag=f"pb{ci}", bufs=1)
            t2 = psmall.tile([P, 1], f32, tag="t2")
            nc.vector.tensor_scalar(t2, mv[:, 0:1], a_col, None, op0=OP.mult)
            nc.vector.tensor_scalar(pb[:, 0:1], t2, float(N), None, op0=OP.mult)
            nc.vector.tensor_copy(pb[:, 1:2], b_col)
            pb_sb.append(pb)

        # ---------------- matvec rows for the rank-1 corrections ----------
        # [qp; qb] rows, then linear-combine into the 4-row rank-1 stacks
        # with tiny constant matmuls (engines can't write partition offsets).
        qrows_p = ps_big.tile([2, C], f32, tag="big")
        for ci in range(CT):
            nc.tensor.matmul(qrows_p, lhsT=pb_sb[ci], rhs=wq_sb[ci],
                             start=(ci == 0), stop=(ci == CT - 1))
        qr2 = pmats.tile([2, C], bf, tag="qr2")
        nc.vector.tensor_copy(qr2, qrows_p)
        krows_p = ps_big.tile([2, C], f32, tag="big")
        for ci in range(CT):
            nc.tensor.matmul(krows_p, lhsT=pb_sb[ci], rhs=wk_sb[ci],
                             start=(ci == 0), stop=(ci == CT - 1))
        kr2 = pmats.tile([2, C], bf, tag="kr2")
        nc.vector.tensor_copy(kr2, krows_p)

        rq_p = ps_big.tile([4, C], f32, tag="big")
        nc.tensor.matmul(rq_p, lhsT=lq2, rhs=qr2, start=True, stop=False)
        nc.tensor.matmul(rq_p, lhsT=lq1, rhs=bqr, start=False, stop=True)
        rows_q = pmats.tile([4, C], bf, tag="rows_q")
        nc.vector.tensor_copy(rows_q, rq_p)

        rk_p = ps_big.tile([4, C], f32, tag="big")
        nc.tensor.matmul(rk_p, lhsT=lk2, rhs=kr2, start=True, stop=False)
        nc.tensor.matmul(rk_p, lhsT=lk1, rhs=bkr, start=False, stop=True)
        rows_k = pmats.tile([4, C], bf, tag="rows_k")
        nc.vector.tensor_copy(rows_k, rk_p)

        vrow_p = ps_big.tile([1, C], f32, tag="big")
        for ci in range(CT):
            nc.tensor.matmul(vrow_p, lhsT=beta_sb[ci], rhs=wvT_sb[ci],
                             start=(ci == 0), stop=(ci == CT - 1))
        vbrow = pmats.tile([1, C], bf, tag="vbrow")
        nc.vector.tensor_tensor(vbrow, vrow_p, bvr, OP.add)
        # vb as per-tile columns (via 1-col transpose matmuls)
        vb_cols = []
        for dt in range(CT):
            cp = ps_big.tile([P, 1], f32, tag="big")
            nc.tensor.matmul(cp, lhsT=vbrow[0:1, dt * P:(dt + 1) * P], rhs=one11,
                             start=True, stop=True)
            vb_c = psmall.tile([P, 1], bf, tag=f"vb{dt}", bufs=1)
            nc.vector.tensor_copy(vb_c, cp)
            vb_cols.append(vb_c)

        # awkT = a * Wk^T (row-scaled)
        awk_sb = []
        for ci in range(CT):
            awk = pmats.tile([P, C], bf, tag=f"awk{ci}")
            nc.vector.tensor_scalar(awk, wk_sb[ci], a_sb[ci], None, op0=OP.mult)
            awk_sb.append(awk)

        # ---------------- Gs = a*Gx evac; A^T = Gs^T Wq^T ----------------
        Gs_sb = []
        for it in range(CT):
            Gs = pmats.tile([P, C], bf, tag=f"Gs{it}")
            nc.vector.tensor_scalar(Gs, G_ps[it], a_sb[it], None, op0=OP.mult)
            Gs_sb.append(Gs)

        AT_sb = []
        for jt in range(CT):
            Ap = ps_big.tile([P, C], f32, tag="big")
            for it in range(CT):
                nc.tensor.matmul(Ap, lhsT=Gs_sb[it][:, jt * P:(jt + 1) * P],
                                 rhs=wq_sb[it], start=(it == 0), stop=(it == CT - 1))
            AT_t = pmats.tile([P, C], bf, tag=f"AT{jt}", name=f"ATsb{jt}")
            nc.vector.tensor_copy(AT_t, Ap)
            AT_sb.append(AT_t)

        # ---------------- scores + softmax ----------------
        pr_sb = []
        for ct in range(CT):
            scp = ps_big.tile([P, C], f32, tag="big")
            for jt in range(CT):
                nc.tensor.matmul(scp, lhsT=AT_sb[jt][:, ct * P:(ct + 1) * P],
                                 rhs=awk_sb[jt], start=(jt == 0), stop=False)
            nc.tensor.matmul(scp, lhsT=rows_q[:, ct * P:(ct + 1) * P], rhs=rows_k,
                             start=False, stop=True)
            nm = psmall.tile([P, 1], f32, tag="nm")
            nc.vector.reduce_max(nm, scp, axis=AX.X, negate=True)
            nma = psmall.tile([P, 1], f32, tag="nma")
            nc.vector.tensor_scalar(nma, nm, ALPHA, None, op0=OP.mult)
            se = psmall.tile([P, 1], f32, tag="se")
            pr_t = pmats.tile([P, C], bf, tag=f"pr{ct}")
            nc.scalar.activation(pr_t, scp, AF.Exp, bias=nma, scale=ALPHA,
                                 accum_out=se)
            ri = psmall.tile([P, 1], f32, tag="ri")
            nc.vector.reciprocal(ri, se)
            nc.vector.tensor_scalar_mul(pr_t, pr_t, ri)
            pr_sb.append(pr_t)

        # probs^T via PE transposes
        ps_tr = ps_ctx.enter_context(tc.tile_pool(name="ps_tr", bufs=2, space="PSUM"))
        prT_sb = [pmats.tile([P, C], bf, tag=f"prT{dt}", name=f"prTsb{dt}")
                  for dt in range(CT)]
        for ct in range(CT):
            trp = ps_tr.tile([P, C], bf, tag="tr")
            for dt in range(CT):
                nc.tensor.transpose(trp[:, dt * P:(dt + 1) * P],
                                    pr_sb[ct][:, dt * P:(dt + 1) * P], ident)
            for dt in range(CT):
                nc.vector.tensor_copy(prT_sb[dt][:, ct * P:(ct + 1) * P],
                                      trp[:, dt * P:(dt + 1) * P])

        # ---------------- M^T (fp8, a-scaled, DoubleRow layout) ----------
        MT2_sb = [pmats.tile([P, 2, C], f8, tag=f"MT2{t}", name=f"MT2sb{t}")
                  for t in range(2)]
        for it in range(CT):
            Mp = ps_big.tile([P, C], f32, tag="big")
            for dt in range(CT):
                nc.tensor.matmul(Mp, lhsT=wv_sb[dt][:, it * P:(it + 1) * P],
                                 rhs=prT_sb[dt], start=(dt == 0), stop=(dt == CT - 1))
            nc.vector.tensor_scalar(MT2_sb[it // 2][:, it % 2, :], Mp, a_sb[it],
                                    None, op0=OP.mult)

        pvp = ps_big.tile([1, C], f32, tag="big")
        for dt in range(CT):
            nc.tensor.matmul(pvp, lhsT=vb_cols[dt], rhs=prT_sb[dt],
                             start=(dt == 0), stop=(dt == CT - 1))
        pv2 = pmats.tile([1, 2, C], f8, tag="pv2")
        nc.vector.tensor_scalar(pv2[:, 0, :], pvp, 2.0, None, op0=OP.mult)
        nc.vector.tensor_scalar(pv2[:, 1, :], pvp, 2.0, None, op0=OP.mult)

        # ---------------- attn^T + residual + store ----------------
        ps_ctx.close()  # release gram banks
        ps_att = ctx.enter_context(tc.tile_pool(name="ps_att", bufs=4, space="PSUM"))
        oqv = oq.rearrange("q (i p) -> q i p", i=2)
        for nt in range(NT):
            at = ps_att.tile([P, C], f32, tag="att", name=f"at{nt}")
            for t in range(2):
                nc.tensor.matmul(at, lhsT=xf2_sb[t][:, :, nt * P:(nt + 1) * P],
                                 rhs=MT2_sb[t], start=(t == 0), stop=False,
                                 perf_mode=PM.DoubleRow)
            nc.tensor.matmul(at, lhsT=oqv, rhs=pv2, start=False, stop=True,
                             perf_mode=PM.DoubleRow)

            resid = presid.tile([P, C], bf, tag="resid")
            ci, lo = nt // 8, nt % 8
            nc.scalar.dma_start(
                resid,
                x_sb[ci][16 * lo:16 * (lo + 1), :].rearrange(
                    "p (u f) -> p u f", u=8),
            )
            osb = pout.tile([P, C], f32, tag="o")
            nc.vector.tensor_tensor(osb, at, resid, OP.add)
            nc.sync.dma_start(out_d[nt * P:(nt + 1) * P, :], osb)

    nc.compile()
    return nc


_NC = None


def _get_program():
    global _NC
    if _NC is None:
        _NC = _build_program()
    return _NC


def _stage_inputs(x, gn_w, gn_b, wq, bq, wk, bk, wv, bv):
    """Host-side sharding + layout/dtype staging (per-core input maps)."""
    x = np.asarray(x, dtype=np.float32).reshape(B, C, N)
    shared = {
        "wqT": np.ascontiguousarray(np.asarray(wq, np.float32).T).astype(BF16),
        "wkT": np.ascontiguousarray(np.asarray(wk, np.float32).T).astype(BF16),
        "wv": np.ascontiguousarray(np.asarray(wv, np.float32)).astype(BF16),
        "wvT": np.ascontiguousarray(np.asarray(wv, np.float32).T).astype(BF16),
        "bq_row": np.asarray(bq, np.float32).reshape(1, C).astype(BF16),
        "bk_row": np.asarray(bk, np.float32).reshape(1, C).astype(BF16),
        "bv_row": np.asarray(bv, np.float32).reshape(1, C),
        # columns: [qp, qb, qs, bq] from rows [qp; qb] (+ bq row)
        "lq2": np.array([[1, 0, 1, 0], [0, 1, N, 0]], np.float32).astype(BF16),
        "lq1": np.array([[0, 0, 0, 1]], np.float32).astype(BF16),
        # columns: [kb, ks, bk, u] from rows [kp; kb] (+ bk row)
        "lk2": np.array([[0, 1, 0, 1], [1, N, 0, N]], np.float32).astype(BF16),
        "lk1": np.array([[0, 0, 1, N]], np.float32).astype(BF16),
        "gnw": np.asarray(gn_w, np.float32).reshape(C, 1),
        "gnb": np.asarray(gn_b, np.float32).reshape(C, 1),
        "oq": np.full((1, 2 * P), 0.25, np.float32).astype(F8NP),
        "one11": np.ones((1, 1), np.float32).astype(BF16),
    }
    ind16 = np.zeros((C, 8), np.float32)
    indT = np.zeros((8, P), np.float32)
    for c in range(C):
        ind16[c, (c % P) // GS] = 1.0 / GS
    for p in range(P):
        indT[p // GS, p] = 1.0
    shared["ind16"] = ind16
    shared["indT01"] = indT

    in_maps = []
    for b in range(B):
        m = dict(shared)
        m["xb"] = np.ascontiguousarray(x[b]).astype(BF16)
        xt = np.ascontiguousarray(x[b].T)                       # (N, C)
        xt2 = (xt.reshape(NPAIR, 2, P, C).transpose(0, 2, 1, 3)
               .reshape(NPAIR * P, 2 * C)).astype(F8NP)
        m["xt2"] = np.ascontiguousarray(xt2)
        in_maps.append(m)
    return in_maps


def kernel(x, gn_w, gn_b, wq, bq, wk, bk, wv, bv, _trace=False, _tmpdir=None):
    nc = _get_program()
    in_maps = _stage_inputs(x, gn_w, gn_b, wq, bq, wk, bk, wv, bv)
    res = bass_utils.run_bass_kernel_spmd(
        nc, in_maps, core_ids=list(range(B)), trace=_trace, tmpdir=_tmpdir,
    )
    out = np.stack([res.results[b]["out"].reshape(C, H, W) for b in range(B)])
    if _trace:
        kernel._last_results = res
    return out.astype(np.float32)
